# revision 5
# baseline (speedup 1.0000x reference)
"""Trainium2 Bass kernel for nn_CrossAttention_29549374997155.

Computation (B=256, U=128, P=64, H=768):
  c[b,u,p] = cosine_sim(u_vec[b,u,:], p_vec[b,p,:])
  row_att = softmax(einsum('bup,oup->bo', c, w_utt) + b_utt)
  col_att = softmax(einsum('bup,opu->bo', c, w_pheno) + b_pheno)

Strategy: pure data parallel over batch (32 batches / core on 8 cores).
Host side: normalize rows (0.1% of FLOPs), lay out transposed operands so
the H contraction sits on SBUF partitions, pre-permute conv weights to
[u, p, o] with row/col output channels concatenated (o = 192), cast to
bf16. Device side per batch: 6 accumulating PE matmuls produce
c = unT.T @ pnT in PSUM; DVE copies it (fp32->bf16) into a persistent
C_all[u, p, batch] tile. Logits for all 32 batches then take 64
accumulating matmuls (contraction chunk = column p of c, stationary
C_all[:, p, :], moving weights [128, 192]) plus one K=1 matmul that adds
the bias via a ones row; the [32, 192] PSUM result has batches on
partitions so both softmaxes run along the free dim.
"""

import sys

if "/opt/trn_rl_repo" not in sys.path:
    sys.path.insert(0, "/opt/trn_rl_repo")

import ml_dtypes
import numpy as np

import concourse.bass as bass  # noqa: F401  (bass registers engine types)
import concourse.tile as tile
from concourse import bacc, mybir
from concourse.bass_utils import run_bass_kernel_spmd

B, U, P, H = 256, 128, 64, 768
NCORES = 8
NB = B // NCORES          # 32 batches per core
HC = H // 128             # 6 contraction chunks
O = U + P                 # 192 fused output channels
GRP = 4                   # batches per input DMA
EPS = 1e-8

IN_DT = mybir.dt.bfloat16
IN_NP = ml_dtypes.bfloat16

_CACHE = {}


def _build():
    nc = bacc.Bacc("TRN2", target_bir_lowering=False, debug=False)

    ut = nc.dram_tensor("ut", [128, NB, HC, U], IN_DT, kind="ExternalInput")
    pt = nc.dram_tensor("pt", [128, NB, HC, P], IN_DT, kind="ExternalInput")
    wt = nc.dram_tensor("wt", [U, P, O], IN_DT, kind="ExternalInput")
    bias = nc.dram_tensor("bias", [1, O], mybir.dt.float32, kind="ExternalInput")
    sel4 = nc.dram_tensor("sel4", [128, NB], mybir.dt.float32, kind="ExternalInput")
    out = nc.dram_tensor("out", [NB, O], mybir.dt.float32, kind="ExternalOutput")

    f32 = mybir.dt.float32

    with tile.TileContext(nc) as tc:
        with (
            tc.tile_pool(name="u_in", bufs=3) as u_pool,
            tc.tile_pool(name="p_in", bufs=3) as p_pool,
            tc.tile_pool(name="singles", bufs=1) as singles,
            tc.tile_pool(name="cps", bufs=4, space="PSUM") as cps_pool,
            tc.tile_pool(name="lps", bufs=1, space="PSUM") as lps_pool,
            tc.tile_pool(name="sm", bufs=1) as sm_pool,
        ):
            # weights/constants go on the scalar HWDGE queue so the input
            # stream on the sync queue starts immediately
            wt_t = singles.tile([U, P, O], IN_DT)
            nc.scalar.dma_start(out=wt_t[:], in_=wt.ap())
            bias_t = singles.tile([1, O], f32)
            nc.scalar.dma_start(out=bias_t[:], in_=bias.ap())
            sel4_t = singles.tile([128, NB], f32)
            nc.scalar.dma_start(out=sel4_t[:], in_=sel4.ap())
            ones_t = singles.tile([1, NB], f32)
            nc.vector.memset(ones_t[:], 1.0)
            # prime the ACT Exp table during the DMA phase (the table load
            # is inserted before the first ACTIVATE in the ACT stream)
            dummy = singles.tile([1, 1], f32)
            nc.vector.memset(dummy[:], 0.0)
            nc.scalar.activation(
                out=dummy[:], in_=dummy[:],
                func=mybir.ActivationFunctionType.Exp,
            )

            c_all = singles.tile([U, P, NB], IN_DT)

            ut_ap = ut.ap()
            pt_ap = pt.ap()
            for g in range(NB // GRP):
                u_t = u_pool.tile([128, GRP, HC, U], IN_DT)
                nc.sync.dma_start(
                    out=u_t[:], in_=ut_ap[:, g * GRP : (g + 1) * GRP, :, :]
                )
                p_t = p_pool.tile([128, GRP, HC, P], IN_DT)
                nc.sync.dma_start(
                    out=p_t[:], in_=pt_ap[:, g * GRP : (g + 1) * GRP, :, :]
                )
                for jj in range(GRP):
                    j = g * GRP + jj
                    ps_c = cps_pool.tile([U, P], f32)
                    for c in range(HC):
                        nc.tensor.matmul(
                            ps_c[:],
                            lhsT=u_t[:, jj, c, :],
                            rhs=p_t[:, jj, c, :],
                            start=(c == 0),
                            stop=(c == HC - 1),
                        )
                    nc.vector.tensor_copy(out=c_all[:, :, j], in_=ps_c[:])

            # logits: 64 K=128 chunks, 4 packed per PE pass via column tiling
            # (chunk p -> array columns 32*(p%4) .. +32, psum rows 32*(p%4)..)
            ps4 = lps_pool.tile([128, O], f32)
            for p in range(P):
                t = p % 4
                nc.tensor.matmul(
                    ps4[32 * t : 32 * (t + 1), :],
                    lhsT=c_all[:, p, :],
                    rhs=wt_t[:, p, :],
                    start=(p < 4),
                    stop=(p >= P - 4),
                    tile_position=(0, 32 * t),
                )
            s4 = sm_pool.tile([128, O], f32)
            nc.vector.tensor_copy(out=s4[:], in_=ps4[:])
            # reduce the 4 partial blocks (fp32 matmul with a stacked
            # identity) and add the bias via a ones row
            ps_l = lps_pool.tile([NB, O], f32)
            nc.tensor.matmul(
                ps_l[:], lhsT=sel4_t[:], rhs=s4[:], start=True, stop=False
            )
            nc.tensor.matmul(
                ps_l[:], lhsT=ones_t[:], rhs=bias_t[:], start=False, stop=True
            )

            # two softmaxes along the free dim: [:, :U] rows, [:, U:] cols
            e_t = sm_pool.tile([NB, O], f32)
            out_t = sm_pool.tile([NB, O], f32)
            neg_m = {}
            for lo, hi in ((0, U), (U, O)):
                neg_m[lo] = sm_pool.tile([NB, 1], f32, name=f"negm{lo}", tag=f"negm{lo}")
                nc.vector.reduce_max(
                    out=neg_m[lo][:], in_=ps_l[:, lo:hi],
                    axis=mybir.AxisListType.X, negate=True,
                )
            s_e = {}
            for lo, hi in ((0, U), (U, O)):
                s_e[lo] = sm_pool.tile([NB, 1], f32, name=f"sume{lo}", tag=f"sume{lo}")
                nc.scalar.activation(
                    out=e_t[:, lo:hi], in_=ps_l[:, lo:hi],
                    func=mybir.ActivationFunctionType.Exp,
                    bias=neg_m[lo][:], scale=1.0, accum_out=s_e[lo][:],
                )
            for lo, hi in ((0, U), (U, O)):
                r_e = sm_pool.tile([NB, 1], f32, name=f"rece{lo}", tag=f"rece{lo}")
                nc.vector.reciprocal(out=r_e[:], in_=s_e[lo][:])
                nc.vector.tensor_scalar_mul(
                    out=out_t[:, lo:hi], in0=e_t[:, lo:hi], scalar1=r_e[:]
                )
            nc.sync.dma_start(out=out.ap(), in_=out_t[:])

    nc.compile()
    return nc


def _prep(utt_output, pheno_output, w_utt, b_utt, w_pheno, b_pheno):
    """Normalize, transpose and shard inputs on the host."""
    u = np.ascontiguousarray(np.swapaxes(np.asarray(utt_output), 0, 1))  # [B, U, H]
    p = np.ascontiguousarray(np.swapaxes(np.asarray(pheno_output), 0, 1))  # [B, P, H]
    un = u / np.maximum(np.linalg.norm(u, axis=-1, keepdims=True), EPS)
    pn = p / np.maximum(np.linalg.norm(p, axis=-1, keepdims=True), EPS)

    # wt[u, p, :U] = w_utt[o, u, p]; wt[u, p, U:] = w_pheno[o, p, u]
    wr = np.transpose(np.asarray(w_utt), (1, 2, 0))     # [U, P, U]
    wc = np.transpose(np.asarray(w_pheno), (2, 1, 0))   # [U, P, P]
    wt = np.ascontiguousarray(
        np.concatenate([wr, wc], axis=2), dtype=np.float32
    ).astype(IN_NP)
    bias = np.concatenate([np.asarray(b_utt), np.asarray(b_pheno)])
    bias = np.ascontiguousarray(bias.reshape(1, O), dtype=np.float32)
    sel4 = np.ascontiguousarray(
        np.tile(np.eye(NB, dtype=np.float32), (128 // NB, 1))
    )

    in_maps = []
    for i in range(NCORES):
        j0 = i * NB
        # [NB, U, H] -> [NB, U, HC, 128] -> (h_lo, j, c, u)
        ut_i = (
            un[j0 : j0 + NB]
            .reshape(NB, U, HC, 128)
            .transpose(3, 0, 2, 1)
        )
        pt_i = (
            pn[j0 : j0 + NB]
            .reshape(NB, P, HC, 128)
            .transpose(3, 0, 2, 1)
        )
        in_maps.append(
            {
                "ut": np.ascontiguousarray(ut_i, dtype=np.float32).astype(IN_NP),
                "pt": np.ascontiguousarray(pt_i, dtype=np.float32).astype(IN_NP),
                "wt": wt,
                "bias": bias,
                "sel4": sel4,
            }
        )
    return in_maps


def _run(inputs, trace=False, trace_cores=None):
    if "nc" not in _CACHE:
        _CACHE["nc"] = _build()
    nc = _CACHE["nc"]
    in_maps = _prep(**inputs)
    res = run_bass_kernel_spmd(
        nc, in_maps, core_ids=list(range(NCORES)), trace=trace,
        trace_cores=trace_cores,
    )
    outs = [res.results[i]["out"] for i in range(NCORES)]
    row = np.concatenate([o[:, :U] for o in outs], axis=0).astype(np.float32)
    col = np.concatenate([o[:, U:] for o in outs], axis=0).astype(np.float32)
    return (row, col), res


def kernel(**inputs):
    (row, col), _ = _run(inputs, trace=False)
    return row, col


# revision 8
# speedup vs baseline: 1.2737x; 1.2737x over previous
"""Trainium2 Bass kernel for nn_CrossAttention_29549374997155.

Computation (B=256, U=128, P=64, H=768):
  c[b,u,p] = cosine_sim(u_vec[b,u,:], p_vec[b,p,:])
  row_att = softmax(einsum('bup,oup->bo', c, w_utt) + b_utt)
  col_att = softmax(einsum('bup,opu->bo', c, w_pheno) + b_pheno)

Strategy: pure data parallel over batch (32 batches / core on 8 cores).
Host side: normalize rows (0.1% of FLOPs), lay out transposed operands so
the H contraction sits on SBUF partitions, pre-permute conv weights to
[u, p, o] with row/col output channels concatenated (o = 192), cast to
bf16. Device side per batch: 6 accumulating PE matmuls produce
c = unT.T @ pnT in PSUM; DVE copies it (fp32->bf16) into a persistent
C_all[u, p, batch] tile. Logits for all 32 batches then take 64
accumulating matmuls (contraction chunk = column p of c, stationary
C_all[:, p, :], moving weights [128, 192]) plus one K=1 matmul that adds
the bias via a ones row; the [32, 192] PSUM result has batches on
partitions so both softmaxes run along the free dim.
"""

import sys

if "/opt/trn_rl_repo" not in sys.path:
    sys.path.insert(0, "/opt/trn_rl_repo")

import ml_dtypes
import numpy as np

import concourse.bass as bass  # noqa: F401  (bass registers engine types)
import concourse.tile as tile
from concourse import bacc, mybir
from concourse.bass_utils import run_bass_kernel_spmd

B, U, P, H = 256, 128, 64, 768
NCORES = 8
NB = B // NCORES          # 32 batches per core
HC = H // 128             # 6 contraction chunks
O = U + P                 # 192 fused output channels
GRP = 4                   # batches per input DMA
EPS = 1e-8

import os

# activation (u/p) stream dtype: fp8 e4m3 halves the DMA footprint vs bf16.
# Values are pre-scaled by ACT_SCALE on the host so they sit in e4m3's
# normal range; the combined ACT_SCALE^2 factor is divided back out of the
# conv weights (logits are linear in c).
FP8 = os.environ.get("KERNEL_FP8", "1") == "1"
if FP8:
    AB_DT = mybir.dt.float8e4
    AB_NP = ml_dtypes.float8_e4m3fn
    ACT_SCALE = 32.0
else:
    AB_DT = mybir.dt.bfloat16
    AB_NP = ml_dtypes.bfloat16
    ACT_SCALE = 1.0

IN_DT = mybir.dt.bfloat16
IN_NP = ml_dtypes.bfloat16

_CACHE = {}


def _build():
    nc = bacc.Bacc("TRN2", target_bir_lowering=False, debug=False)

    ut = nc.dram_tensor("ut", [128, NB, HC, U], AB_DT, kind="ExternalInput")
    pt = nc.dram_tensor("pt", [128, NB, HC, P], AB_DT, kind="ExternalInput")
    wt = nc.dram_tensor("wt", [U, P, O], IN_DT, kind="ExternalInput")
    bias = nc.dram_tensor("bias", [1, O], mybir.dt.float32, kind="ExternalInput")
    sel4 = nc.dram_tensor("sel4", [128, NB], mybir.dt.float32, kind="ExternalInput")
    out = nc.dram_tensor("out", [NB, O], mybir.dt.float32, kind="ExternalOutput")

    f32 = mybir.dt.float32

    with tile.TileContext(nc) as tc:
        with (
            tc.tile_pool(name="u_in", bufs=3) as u_pool,
            tc.tile_pool(name="p_in", bufs=3) as p_pool,
            tc.tile_pool(name="singles", bufs=1) as singles,
            tc.tile_pool(name="cps", bufs=4, space="PSUM") as cps_pool,
            tc.tile_pool(name="lps", bufs=1, space="PSUM") as lps_pool,
            tc.tile_pool(name="sm", bufs=1) as sm_pool,
        ):
            # weights/constants go on the scalar HWDGE queue so the input
            # stream on the sync queue starts immediately
            wt_t = singles.tile([U, P, O], IN_DT)
            nc.scalar.dma_start(out=wt_t[:], in_=wt.ap())
            bias_t = singles.tile([1, O], f32)
            nc.scalar.dma_start(out=bias_t[:], in_=bias.ap())
            sel4_t = singles.tile([128, NB], f32)
            nc.scalar.dma_start(out=sel4_t[:], in_=sel4.ap())
            ones_t = singles.tile([1, NB], f32)
            nc.vector.memset(ones_t[:], 1.0)
            # prime the ACT Exp table during the DMA phase (the table load
            # is inserted before the first ACTIVATE in the ACT stream)
            dummy = singles.tile([1, 1], f32)
            nc.vector.memset(dummy[:], 0.0)
            nc.scalar.activation(
                out=dummy[:], in_=dummy[:],
                func=mybir.ActivationFunctionType.Exp,
            )

            c_all = singles.tile([U, P, NB], IN_DT)

            ut_ap = ut.ap()
            pt_ap = pt.ap()
            for g in range(NB // GRP):
                u_t = u_pool.tile([128, GRP, HC, U], AB_DT)
                nc.sync.dma_start(
                    out=u_t[:], in_=ut_ap[:, g * GRP : (g + 1) * GRP, :, :]
                )
                p_t = p_pool.tile([128, GRP, HC, P], AB_DT)
                nc.sync.dma_start(
                    out=p_t[:], in_=pt_ap[:, g * GRP : (g + 1) * GRP, :, :]
                )
                for jj in range(GRP):
                    j = g * GRP + jj
                    ps_c = cps_pool.tile([U, P], f32)
                    for c in range(HC):
                        nc.tensor.matmul(
                            ps_c[:],
                            lhsT=u_t[:, jj, c, :],
                            rhs=p_t[:, jj, c, :],
                            start=(c == 0),
                            stop=(c == HC - 1),
                        )
                    nc.vector.tensor_copy(out=c_all[:, :, j], in_=ps_c[:])

            # logits: 64 K=128 chunks, 4 packed per PE pass via column tiling
            # (chunk p -> array columns 32*(p%4) .. +32, psum rows 32*(p%4)..)
            ps4 = lps_pool.tile([128, O], f32)
            for p in range(P):
                t = p % 4
                nc.tensor.matmul(
                    ps4[32 * t : 32 * (t + 1), :],
                    lhsT=c_all[:, p, :],
                    rhs=wt_t[:, p, :],
                    start=(p < 4),
                    stop=(p >= P - 4),
                    tile_position=(0, 32 * t),
                )
            s4 = sm_pool.tile([128, O], f32)
            nc.vector.tensor_copy(out=s4[:], in_=ps4[:])
            # reduce the 4 partial blocks (fp32 matmul with a stacked
            # identity) and add the bias via a ones row
            ps_l = lps_pool.tile([NB, O], f32)
            nc.tensor.matmul(
                ps_l[:], lhsT=sel4_t[:], rhs=s4[:], start=True, stop=False
            )
            nc.tensor.matmul(
                ps_l[:], lhsT=ones_t[:], rhs=bias_t[:], start=False, stop=True
            )

            # two softmaxes along the free dim: [:, :U] rows, [:, U:] cols
            e_t = sm_pool.tile([NB, O], f32)
            out_t = sm_pool.tile([NB, O], f32)
            neg_m = {}
            for lo, hi in ((0, U), (U, O)):
                neg_m[lo] = sm_pool.tile([NB, 1], f32, name=f"negm{lo}", tag=f"negm{lo}")
                nc.vector.reduce_max(
                    out=neg_m[lo][:], in_=ps_l[:, lo:hi],
                    axis=mybir.AxisListType.X, negate=True,
                )
            s_e = {}
            for lo, hi in ((0, U), (U, O)):
                s_e[lo] = sm_pool.tile([NB, 1], f32, name=f"sume{lo}", tag=f"sume{lo}")
                nc.scalar.activation(
                    out=e_t[:, lo:hi], in_=ps_l[:, lo:hi],
                    func=mybir.ActivationFunctionType.Exp,
                    bias=neg_m[lo][:], scale=1.0, accum_out=s_e[lo][:],
                )
            for lo, hi in ((0, U), (U, O)):
                r_e = sm_pool.tile([NB, 1], f32, name=f"rece{lo}", tag=f"rece{lo}")
                nc.vector.reciprocal(out=r_e[:], in_=s_e[lo][:])
                nc.vector.tensor_scalar_mul(
                    out=out_t[:, lo:hi], in0=e_t[:, lo:hi], scalar1=r_e[:]
                )
            nc.sync.dma_start(out=out.ap(), in_=out_t[:])

    nc.compile()
    return nc


def _prep(utt_output, pheno_output, w_utt, b_utt, w_pheno, b_pheno):
    """Normalize, transpose and shard inputs on the host."""
    u = np.ascontiguousarray(np.swapaxes(np.asarray(utt_output), 0, 1))  # [B, U, H]
    p = np.ascontiguousarray(np.swapaxes(np.asarray(pheno_output), 0, 1))  # [B, P, H]
    un = u * (ACT_SCALE / np.maximum(np.linalg.norm(u, axis=-1, keepdims=True), EPS))
    pn = p * (ACT_SCALE / np.maximum(np.linalg.norm(p, axis=-1, keepdims=True), EPS))

    # wt[u, p, :U] = w_utt[o, u, p]; wt[u, p, U:] = w_pheno[o, p, u]
    wr = np.transpose(np.asarray(w_utt), (1, 2, 0))     # [U, P, U]
    wc = np.transpose(np.asarray(w_pheno), (2, 1, 0))   # [U, P, P]
    wt = np.ascontiguousarray(
        np.concatenate([wr, wc], axis=2), dtype=np.float32
    )
    wt = (wt / (ACT_SCALE * ACT_SCALE)).astype(IN_NP)
    bias = np.concatenate([np.asarray(b_utt), np.asarray(b_pheno)])
    bias = np.ascontiguousarray(bias.reshape(1, O), dtype=np.float32)
    sel4 = np.ascontiguousarray(
        np.tile(np.eye(NB, dtype=np.float32), (128 // NB, 1))
    )

    in_maps = []
    for i in range(NCORES):
        j0 = i * NB
        # [NB, U, H] -> [NB, U, HC, 128] -> (h_lo, j, c, u)
        ut_i = (
            un[j0 : j0 + NB]
            .reshape(NB, U, HC, 128)
            .transpose(3, 0, 2, 1)
        )
        pt_i = (
            pn[j0 : j0 + NB]
            .reshape(NB, P, HC, 128)
            .transpose(3, 0, 2, 1)
        )
        in_maps.append(
            {
                "ut": np.ascontiguousarray(ut_i, dtype=np.float32).astype(AB_NP),
                "pt": np.ascontiguousarray(pt_i, dtype=np.float32).astype(AB_NP),
                "wt": wt,
                "bias": bias,
                "sel4": sel4,
            }
        )
    return in_maps


def _run(inputs, trace=False, trace_cores=None):
    if "nc" not in _CACHE:
        _CACHE["nc"] = _build()
    nc = _CACHE["nc"]
    in_maps = _prep(**inputs)
    res = run_bass_kernel_spmd(
        nc, in_maps, core_ids=list(range(NCORES)), trace=trace,
        trace_cores=trace_cores,
    )
    outs = [res.results[i]["out"] for i in range(NCORES)]
    row = np.concatenate([o[:, :U] for o in outs], axis=0).astype(np.float32)
    col = np.concatenate([o[:, U:] for o in outs], axis=0).astype(np.float32)
    return (row, col), res


def kernel(**inputs):
    (row, col), _ = _run(inputs, trace=False)
    return row, col


# revision 9
# speedup vs baseline: 1.5153x; 1.1896x over previous
"""Trainium2 Bass kernel for nn_CrossAttention_29549374997155.

Computation (B=256, U=128, P=64, H=768):
  c[b,u,p] = cosine_sim(u_vec[b,u,:], p_vec[b,p,:])
  row_att = softmax(einsum('bup,oup->bo', c, w_utt) + b_utt)
  col_att = softmax(einsum('bup,opu->bo', c, w_pheno) + b_pheno)

Strategy: pure data parallel over batch (32 batches / core on 8 cores).
Host side: normalize rows (0.1% of FLOPs), lay out transposed operands so
the H contraction sits on SBUF partitions, pre-permute conv weights to
[u, p, o] with row/col output channels concatenated (o = 192), cast to
bf16. Device side per batch: 6 accumulating PE matmuls produce
c = unT.T @ pnT in PSUM; DVE copies it (fp32->bf16) into a persistent
C_all[u, p, batch] tile. Logits for all 32 batches then take 64
accumulating matmuls (contraction chunk = column p of c, stationary
C_all[:, p, :], moving weights [128, 192]) plus one K=1 matmul that adds
the bias via a ones row; the [32, 192] PSUM result has batches on
partitions so both softmaxes run along the free dim.
"""

import sys

if "/opt/trn_rl_repo" not in sys.path:
    sys.path.insert(0, "/opt/trn_rl_repo")

import ml_dtypes
import numpy as np

import concourse.bass as bass  # noqa: F401  (bass registers engine types)
import concourse.tile as tile
from concourse import bacc, mybir
from concourse.bass_utils import run_bass_kernel_spmd

B, U, P, H = 256, 128, 64, 768
NCORES = 8
NB = B // NCORES          # 32 batches per core
HC = H // 128             # 6 contraction chunks
O = U + P                 # 192 fused output channels
GRP = 4                   # batches per input DMA
EPS = 1e-8

import os

# activation (u/p) stream dtype: fp8 e4m3 halves the DMA footprint vs bf16.
# Values are pre-scaled by ACT_SCALE on the host so they sit in e4m3's
# normal range; the combined ACT_SCALE^2 factor is divided back out of the
# conv weights (logits are linear in c).
FP8 = os.environ.get("KERNEL_FP8", "1") == "1"
if FP8:
    AB_DT = mybir.dt.float8e4
    AB_NP = ml_dtypes.float8_e4m3fn
    ACT_SCALE = 32.0
    W_SCALE = 32.0
else:
    AB_DT = mybir.dt.bfloat16
    AB_NP = ml_dtypes.bfloat16
    ACT_SCALE = 1.0
    W_SCALE = 1.0

IN_DT = mybir.dt.bfloat16
IN_NP = ml_dtypes.bfloat16

_CACHE = {}


def _build():
    nc = bacc.Bacc("TRN2", target_bir_lowering=False, debug=False)

    ut = nc.dram_tensor("ut", [128, NB, HC, U], AB_DT, kind="ExternalInput")
    pt = nc.dram_tensor("pt", [128, NB, HC, P], AB_DT, kind="ExternalInput")
    wt = nc.dram_tensor("wt", [U, P, O], AB_DT, kind="ExternalInput")
    bias = nc.dram_tensor("bias", [1, O], mybir.dt.float32, kind="ExternalInput")
    sel4 = nc.dram_tensor("sel4", [128, NB], mybir.dt.float32, kind="ExternalInput")
    out = nc.dram_tensor("out", [NB, O], mybir.dt.float32, kind="ExternalOutput")

    f32 = mybir.dt.float32

    with tile.TileContext(nc) as tc:
        with (
            tc.tile_pool(name="u_in", bufs=5) as u_pool,
            tc.tile_pool(name="p_in", bufs=5) as p_pool,
            tc.tile_pool(name="singles", bufs=1) as singles,
            tc.tile_pool(name="cps", bufs=4, space="PSUM") as cps_pool,
            tc.tile_pool(name="lps", bufs=1, space="PSUM") as lps_pool,
            tc.tile_pool(name="sm", bufs=1) as sm_pool,
        ):
            # weights/constants go on the scalar HWDGE queue so the input
            # stream on the sync queue starts immediately
            wt_t = singles.tile([U, P, O], AB_DT)
            nc.scalar.dma_start(out=wt_t[:], in_=wt.ap())
            bias_t = singles.tile([1, O], f32)
            nc.scalar.dma_start(out=bias_t[:], in_=bias.ap())
            sel4_t = singles.tile([128, NB], f32)
            nc.scalar.dma_start(out=sel4_t[:], in_=sel4.ap())
            ones_t = singles.tile([1, NB], f32)
            nc.vector.memset(ones_t[:], 1.0)
            # prime the ACT Exp table during the DMA phase (the table load
            # is inserted before the first ACTIVATE in the ACT stream)
            dummy = singles.tile([1, 1], f32)
            nc.vector.memset(dummy[:], 0.0)
            nc.scalar.activation(
                out=dummy[:], in_=dummy[:],
                func=mybir.ActivationFunctionType.Exp,
            )

            c_all = singles.tile([U, P, NB], IN_DT)

            ut_ap = ut.ap()
            pt_ap = pt.ap()
            for g in range(NB // GRP):
                u_t = u_pool.tile([128, GRP, HC, U], AB_DT)
                nc.sync.dma_start(
                    out=u_t[:], in_=ut_ap[:, g * GRP : (g + 1) * GRP, :, :]
                )
                p_t = p_pool.tile([128, GRP, HC, P], AB_DT)
                nc.sync.dma_start(
                    out=p_t[:], in_=pt_ap[:, g * GRP : (g + 1) * GRP, :, :]
                )
                for jj in range(GRP):
                    j = g * GRP + jj
                    ps_c = cps_pool.tile([U, P], f32)
                    for c in range(HC):
                        nc.tensor.matmul(
                            ps_c[:],
                            lhsT=u_t[:, jj, c, :],
                            rhs=p_t[:, jj, c, :],
                            start=(c == 0),
                            stop=(c == HC - 1),
                        )
                    nc.vector.tensor_copy(out=c_all[:, :, j], in_=ps_c[:])

            # logits: 64 K=128 chunks, 4 packed per PE pass via column tiling
            # (chunk p -> array columns 32*(p%4) .. +32, psum rows 32*(p%4)..)
            ps4 = lps_pool.tile([128, O], f32)
            for p in range(P):
                t = p % 4
                nc.tensor.matmul(
                    ps4[32 * t : 32 * (t + 1), :],
                    lhsT=c_all[:, p, :],
                    rhs=wt_t[:, p, :],
                    start=(p < 4),
                    stop=(p >= P - 4),
                    tile_position=(0, 32 * t),
                )
            s4 = sm_pool.tile([128, O], f32)
            nc.vector.tensor_copy(out=s4[:], in_=ps4[:])
            # reduce the 4 partial blocks (fp32 matmul with a stacked
            # identity) and add the bias via a ones row
            ps_l = lps_pool.tile([NB, O], f32)
            nc.tensor.matmul(
                ps_l[:], lhsT=sel4_t[:], rhs=s4[:], start=True, stop=False
            )
            nc.tensor.matmul(
                ps_l[:], lhsT=ones_t[:], rhs=bias_t[:], start=False, stop=True
            )

            # two softmaxes along the free dim: [:, :U] rows, [:, U:] cols
            e_t = sm_pool.tile([NB, O], f32)
            out_t = sm_pool.tile([NB, O], f32)
            neg_m = {}
            for lo, hi in ((0, U), (U, O)):
                neg_m[lo] = sm_pool.tile([NB, 1], f32, name=f"negm{lo}", tag=f"negm{lo}")
                nc.vector.reduce_max(
                    out=neg_m[lo][:], in_=ps_l[:, lo:hi],
                    axis=mybir.AxisListType.X, negate=True,
                )
            s_e = {}
            for lo, hi in ((0, U), (U, O)):
                s_e[lo] = sm_pool.tile([NB, 1], f32, name=f"sume{lo}", tag=f"sume{lo}")
                nc.scalar.activation(
                    out=e_t[:, lo:hi], in_=ps_l[:, lo:hi],
                    func=mybir.ActivationFunctionType.Exp,
                    bias=neg_m[lo][:], scale=1.0, accum_out=s_e[lo][:],
                )
            for lo, hi in ((0, U), (U, O)):
                r_e = sm_pool.tile([NB, 1], f32, name=f"rece{lo}", tag=f"rece{lo}")
                nc.vector.reciprocal(out=r_e[:], in_=s_e[lo][:])
                nc.vector.tensor_scalar_mul(
                    out=out_t[:, lo:hi], in0=e_t[:, lo:hi], scalar1=r_e[:]
                )
            nc.sync.dma_start(out=out.ap(), in_=out_t[:])

    nc.compile()
    return nc


def _prep(utt_output, pheno_output, w_utt, b_utt, w_pheno, b_pheno):
    """Normalize, transpose and shard inputs on the host."""
    u = np.ascontiguousarray(np.swapaxes(np.asarray(utt_output), 0, 1))  # [B, U, H]
    p = np.ascontiguousarray(np.swapaxes(np.asarray(pheno_output), 0, 1))  # [B, P, H]
    un = u * (ACT_SCALE / np.maximum(np.linalg.norm(u, axis=-1, keepdims=True), EPS))
    pn = p * (ACT_SCALE / np.maximum(np.linalg.norm(p, axis=-1, keepdims=True), EPS))

    # wt[u, p, :U] = w_utt[o, u, p]; wt[u, p, U:] = w_pheno[o, p, u]
    wr = np.transpose(np.asarray(w_utt), (1, 2, 0))     # [U, P, U]
    wc = np.transpose(np.asarray(w_pheno), (2, 1, 0))   # [U, P, P]
    wt = np.ascontiguousarray(
        np.concatenate([wr, wc], axis=2), dtype=np.float32
    )
    if FP8:
        wt = (wt * W_SCALE).astype(AB_NP)
    else:
        wt = (wt / (ACT_SCALE * ACT_SCALE)).astype(IN_NP)
    bias = np.concatenate([np.asarray(b_utt), np.asarray(b_pheno)])
    bias = np.ascontiguousarray(bias.reshape(1, O), dtype=np.float32)
    sel4 = np.ascontiguousarray(
        np.tile(np.eye(NB, dtype=np.float32), (128 // NB, 1))
        / (ACT_SCALE * ACT_SCALE * W_SCALE)
    )

    in_maps = []
    for i in range(NCORES):
        j0 = i * NB
        # [NB, U, H] -> [NB, U, HC, 128] -> (h_lo, j, c, u)
        ut_i = (
            un[j0 : j0 + NB]
            .reshape(NB, U, HC, 128)
            .transpose(3, 0, 2, 1)
        )
        pt_i = (
            pn[j0 : j0 + NB]
            .reshape(NB, P, HC, 128)
            .transpose(3, 0, 2, 1)
        )
        in_maps.append(
            {
                "ut": np.ascontiguousarray(ut_i, dtype=np.float32).astype(AB_NP),
                "pt": np.ascontiguousarray(pt_i, dtype=np.float32).astype(AB_NP),
                "wt": wt,
                "bias": bias,
                "sel4": sel4,
            }
        )
    return in_maps


def _run(inputs, trace=False, trace_cores=None):
    if "nc" not in _CACHE:
        _CACHE["nc"] = _build()
    nc = _CACHE["nc"]
    in_maps = _prep(**inputs)
    res = run_bass_kernel_spmd(
        nc, in_maps, core_ids=list(range(NCORES)), trace=trace,
        trace_cores=trace_cores,
    )
    outs = [res.results[i]["out"] for i in range(NCORES)]
    row = np.concatenate([o[:, :U] for o in outs], axis=0).astype(np.float32)
    col = np.concatenate([o[:, U:] for o in outs], axis=0).astype(np.float32)
    return (row, col), res


def kernel(**inputs):
    (row, col), _ = _run(inputs, trace=False)
    return row, col


# revision 10
# speedup vs baseline: 1.5587x; 1.0286x over previous
"""Trainium2 Bass kernel for nn_CrossAttention_29549374997155.

Computation (B=256, U=128, P=64, H=768):
  c[b,u,p] = cosine_sim(u_vec[b,u,:], p_vec[b,p,:])
  row_att = softmax(einsum('bup,oup->bo', c, w_utt) + b_utt)
  col_att = softmax(einsum('bup,opu->bo', c, w_pheno) + b_pheno)

Strategy: pure data parallel over batch (32 batches / core on 8 cores).
Host side: normalize rows (0.1% of FLOPs), lay out transposed operands so
the H contraction sits on SBUF partitions, pre-permute conv weights to
[u, p, o] with row/col output channels concatenated (o = 192), cast to
bf16. Device side per batch: 6 accumulating PE matmuls produce
c = unT.T @ pnT in PSUM; DVE copies it (fp32->bf16) into a persistent
C_all[u, p, batch] tile. Logits for all 32 batches then take 64
accumulating matmuls (contraction chunk = column p of c, stationary
C_all[:, p, :], moving weights [128, 192]) plus one K=1 matmul that adds
the bias via a ones row; the [32, 192] PSUM result has batches on
partitions so both softmaxes run along the free dim.
"""

import sys

if "/opt/trn_rl_repo" not in sys.path:
    sys.path.insert(0, "/opt/trn_rl_repo")

import ml_dtypes
import numpy as np

import concourse.bass as bass  # noqa: F401  (bass registers engine types)
import concourse.tile as tile
from concourse import bacc, mybir
from concourse.bass_utils import run_bass_kernel_spmd

B, U, P, H = 256, 128, 64, 768
NCORES = 8
NB = B // NCORES          # 32 batches per core
HC = H // 128             # 6 contraction chunks
O = U + P                 # 192 fused output channels
GRP = 4                   # batches per input DMA
EPS = 1e-8

import os

# activation (u/p) stream dtype: fp8 e4m3 halves the DMA footprint vs bf16.
# Values are pre-scaled by ACT_SCALE on the host so they sit in e4m3's
# normal range; the combined ACT_SCALE^2 factor is divided back out of the
# conv weights (logits are linear in c).
FP8 = os.environ.get("KERNEL_FP8", "1") == "1"
if FP8:
    AB_DT = mybir.dt.float8e4
    AB_NP = ml_dtypes.float8_e4m3fn
    ACT_SCALE = 32.0
    W_SCALE = 32.0
else:
    AB_DT = mybir.dt.bfloat16
    AB_NP = ml_dtypes.bfloat16
    ACT_SCALE = 1.0
    W_SCALE = 1.0

IN_DT = mybir.dt.bfloat16
IN_NP = ml_dtypes.bfloat16

_CACHE = {}


def _build():
    nc = bacc.Bacc("TRN2", target_bir_lowering=False, debug=False)

    up = nc.dram_tensor("up", [128, NB, HC, O], AB_DT, kind="ExternalInput")
    wt = nc.dram_tensor("wt", [U, P, O], AB_DT, kind="ExternalInput")
    bias = nc.dram_tensor("bias", [1, O], mybir.dt.float32, kind="ExternalInput")
    sel4 = nc.dram_tensor("sel4", [128, NB], mybir.dt.float32, kind="ExternalInput")
    out = nc.dram_tensor("out", [NB, O], mybir.dt.float32, kind="ExternalOutput")

    f32 = mybir.dt.float32

    with tile.TileContext(nc) as tc:
        with (
            tc.tile_pool(name="u_in", bufs=5) as u_pool,
            tc.tile_pool(name="singles", bufs=1) as singles,
            tc.tile_pool(name="cps", bufs=4, space="PSUM") as cps_pool,
            tc.tile_pool(name="lps", bufs=1, space="PSUM") as lps_pool,
            tc.tile_pool(name="sm", bufs=1) as sm_pool,
        ):
            # weights/constants go on the scalar HWDGE queue so the input
            # stream on the sync queue starts immediately
            wt_t = singles.tile([U, P, O], AB_DT)
            nc.scalar.dma_start(out=wt_t[:], in_=wt.ap())
            bias_t = singles.tile([1, O], f32)
            nc.scalar.dma_start(out=bias_t[:], in_=bias.ap())
            sel4_t = singles.tile([128, NB], f32)
            nc.scalar.dma_start(out=sel4_t[:], in_=sel4.ap())
            ones_t = singles.tile([1, NB], f32)
            nc.vector.memset(ones_t[:], 1.0)
            # prime the ACT Exp table during the DMA phase (the table load
            # is inserted before the first ACTIVATE in the ACT stream)
            dummy = singles.tile([1, 1], f32)
            nc.vector.memset(dummy[:], 0.0)
            nc.scalar.activation(
                out=dummy[:], in_=dummy[:],
                func=mybir.ActivationFunctionType.Exp,
            )

            c_all = singles.tile([U, NB, P], IN_DT)

            up_ap = up.ap()
            # first groups are small so their DMA-completion sems fire early
            # and the PE pipeline starts; steady state uses GRP batches
            sizes = [1, 3] + [GRP] * ((NB - 4) // GRP)
            j0 = 0
            for gsz in sizes:
                u_t = u_pool.tile([128, gsz, HC, O], AB_DT, name="u_t", tag="u_t")
                nc.sync.dma_start(out=u_t[:], in_=up_ap[:, j0 : j0 + gsz, :, :])
                for jj in range(gsz):
                    j = j0 + jj
                    ps_c = cps_pool.tile([U, P], f32)
                    for c in range(HC):
                        nc.tensor.matmul(
                            ps_c[:],
                            lhsT=u_t[:, jj, c, 0:U],
                            rhs=u_t[:, jj, c, U:O],
                            start=(c == 0),
                            stop=(c == HC - 1),
                        )
                    nc.vector.tensor_copy(out=c_all[:, j, :], in_=ps_c[:])
                j0 += gsz

            # logits: 64 K=128 chunks, 4 packed per PE pass via column tiling
            # (chunk p -> array columns 32*(p%4) .. +32, psum rows 32*(p%4)..)
            ps4 = lps_pool.tile([128, O], f32)
            for p in range(P):
                t = p % 4
                nc.tensor.matmul(
                    ps4[32 * t : 32 * (t + 1), :],
                    lhsT=c_all[:, :, p],
                    rhs=wt_t[:, p, :],
                    start=(p < 4),
                    stop=(p >= P - 4),
                    tile_position=(0, 32 * t),
                )
            s4 = sm_pool.tile([128, O], f32)
            nc.vector.tensor_copy(out=s4[:], in_=ps4[:])
            # reduce the 4 partial blocks (fp32 matmul with a stacked
            # identity) and add the bias via a ones row
            ps_l = lps_pool.tile([NB, O], f32)
            nc.tensor.matmul(
                ps_l[:], lhsT=sel4_t[:], rhs=s4[:], start=True, stop=False
            )
            nc.tensor.matmul(
                ps_l[:], lhsT=ones_t[:], rhs=bias_t[:], start=False, stop=True
            )

            # two softmaxes along the free dim: [:, :U] rows, [:, U:] cols
            e_t = sm_pool.tile([NB, O], f32)
            out_t = sm_pool.tile([NB, O], f32)
            s_e = {}
            for lo, hi in ((0, U), (U, O)):
                s_e[lo] = sm_pool.tile([NB, 1], f32, name=f"sume{lo}", tag=f"sume{lo}")
                nc.scalar.activation(
                    out=e_t[:, lo:hi], in_=ps_l[:, lo:hi],
                    func=mybir.ActivationFunctionType.Exp,
                    bias=0.0, scale=1.0, accum_out=s_e[lo][:],
                )
            for lo, hi in ((0, U), (U, O)):
                r_e = sm_pool.tile([NB, 1], f32, name=f"rece{lo}", tag=f"rece{lo}")
                nc.vector.reciprocal(out=r_e[:], in_=s_e[lo][:])
                nc.vector.tensor_scalar_mul(
                    out=out_t[:, lo:hi], in0=e_t[:, lo:hi], scalar1=r_e[:]
                )
            nc.sync.dma_start(out=out.ap(), in_=out_t[:])

    nc.compile()
    return nc


def _prep(utt_output, pheno_output, w_utt, b_utt, w_pheno, b_pheno):
    """Normalize, transpose and shard inputs on the host."""
    u = np.ascontiguousarray(np.swapaxes(np.asarray(utt_output), 0, 1))  # [B, U, H]
    p = np.ascontiguousarray(np.swapaxes(np.asarray(pheno_output), 0, 1))  # [B, P, H]
    un = u * (ACT_SCALE / np.maximum(np.linalg.norm(u, axis=-1, keepdims=True), EPS))
    pn = p * (ACT_SCALE / np.maximum(np.linalg.norm(p, axis=-1, keepdims=True), EPS))

    # wt[u, p, :U] = w_utt[o, u, p]; wt[u, p, U:] = w_pheno[o, p, u]
    wr = np.transpose(np.asarray(w_utt), (1, 2, 0))     # [U, P, U]
    wc = np.transpose(np.asarray(w_pheno), (2, 1, 0))   # [U, P, P]
    wt = np.ascontiguousarray(
        np.concatenate([wr, wc], axis=2), dtype=np.float32
    )
    if FP8:
        wt = (wt * W_SCALE).astype(AB_NP)
    else:
        wt = (wt / (ACT_SCALE * ACT_SCALE)).astype(IN_NP)
    bias = np.concatenate([np.asarray(b_utt), np.asarray(b_pheno)])
    bias = np.ascontiguousarray(bias.reshape(1, O), dtype=np.float32)
    sel4 = np.ascontiguousarray(
        np.tile(np.eye(NB, dtype=np.float32), (128 // NB, 1))
        / (ACT_SCALE * ACT_SCALE * W_SCALE)
    )

    in_maps = []
    for i in range(NCORES):
        j0 = i * NB
        # [NB, U, H] -> [NB, U, HC, 128] -> (h_lo, j, c, u)
        ut_i = (
            un[j0 : j0 + NB]
            .reshape(NB, U, HC, 128)
            .transpose(3, 0, 2, 1)
        )
        pt_i = (
            pn[j0 : j0 + NB]
            .reshape(NB, P, HC, 128)
            .transpose(3, 0, 2, 1)
        )
        in_maps.append(
            {
                "up": np.ascontiguousarray(
                    np.concatenate([ut_i, pt_i], axis=3), dtype=np.float32
                ).astype(AB_NP),
                "wt": wt,
                "bias": bias,
                "sel4": sel4,
            }
        )
    return in_maps


def _run(inputs, trace=False, trace_cores=None):
    if "nc" not in _CACHE:
        _CACHE["nc"] = _build()
    nc = _CACHE["nc"]
    in_maps = _prep(**inputs)
    res = run_bass_kernel_spmd(
        nc, in_maps, core_ids=list(range(NCORES)), trace=trace,
        trace_cores=trace_cores,
    )
    outs = [res.results[i]["out"] for i in range(NCORES)]
    row = np.concatenate([o[:, :U] for o in outs], axis=0).astype(np.float32)
    col = np.concatenate([o[:, U:] for o in outs], axis=0).astype(np.float32)
    return (row, col), res


def kernel(**inputs):
    (row, col), _ = _run(inputs, trace=False)
    return row, col


# revision 11
# speedup vs baseline: 1.6051x; 1.0298x over previous
"""Trainium2 Bass kernel for nn_CrossAttention_29549374997155.

Computation (B=256, U=128, P=64, H=768):
  c[b,u,p] = cosine_sim(u_vec[b,u,:], p_vec[b,p,:])
  row_att = softmax(einsum('bup,oup->bo', c, w_utt) + b_utt)
  col_att = softmax(einsum('bup,opu->bo', c, w_pheno) + b_pheno)

Strategy: pure data parallel over batch (32 batches / core on 8 cores).
Host side: normalize rows (0.1% of FLOPs), lay out transposed operands so
the H contraction sits on SBUF partitions, pre-permute conv weights to
[u, p, o] with row/col output channels concatenated (o = 192), cast to
bf16. Device side per batch: 6 accumulating PE matmuls produce
c = unT.T @ pnT in PSUM; DVE copies it (fp32->bf16) into a persistent
C_all[u, p, batch] tile. Logits for all 32 batches then take 64
accumulating matmuls (contraction chunk = column p of c, stationary
C_all[:, p, :], moving weights [128, 192]) plus one K=1 matmul that adds
the bias via a ones row; the [32, 192] PSUM result has batches on
partitions so both softmaxes run along the free dim.
"""

import sys

if "/opt/trn_rl_repo" not in sys.path:
    sys.path.insert(0, "/opt/trn_rl_repo")

import ml_dtypes
import numpy as np

import concourse.bass as bass  # noqa: F401  (bass registers engine types)
import concourse.tile as tile
from concourse import bacc, mybir
from concourse.bass_utils import run_bass_kernel_spmd

B, U, P, H = 256, 128, 64, 768
NCORES = 8
NB = B // NCORES          # 32 batches per core
HC = H // 128             # 6 contraction chunks
O = U + P                 # 192 fused output channels
GRP = 4                   # batches per input DMA
EPS = 1e-8

import os

# activation (u/p) stream dtype: fp8 e4m3 halves the DMA footprint vs bf16.
# Values are pre-scaled by ACT_SCALE on the host so they sit in e4m3's
# normal range; the combined ACT_SCALE^2 factor is divided back out of the
# conv weights (logits are linear in c).
FP8 = os.environ.get("KERNEL_FP8", "1") == "1"
if FP8:
    AB_DT = mybir.dt.float8e4
    AB_NP = ml_dtypes.float8_e4m3fn
    ACT_SCALE = 32.0
    W_SCALE = 32.0
else:
    AB_DT = mybir.dt.bfloat16
    AB_NP = ml_dtypes.bfloat16
    ACT_SCALE = 1.0
    W_SCALE = 1.0

IN_DT = mybir.dt.bfloat16
IN_NP = ml_dtypes.bfloat16

_CACHE = {}


def _build():
    nc = bacc.Bacc("TRN2", target_bir_lowering=False, debug=False)

    up = nc.dram_tensor("up", [128, NB, HC, O], AB_DT, kind="ExternalInput")
    wt = nc.dram_tensor("wt", [U, P, O], AB_DT, kind="ExternalInput")
    bias = nc.dram_tensor("bias", [1, O], mybir.dt.float32, kind="ExternalInput")
    sel4 = nc.dram_tensor("sel4", [128, NB], mybir.dt.float32, kind="ExternalInput")
    out = nc.dram_tensor("out", [NB, O], mybir.dt.float32, kind="ExternalOutput")

    f32 = mybir.dt.float32

    with tile.TileContext(nc) as tc:
        with (
            tc.tile_pool(name="u_in", bufs=5) as u_pool,
            tc.tile_pool(name="singles", bufs=1) as singles,
            tc.tile_pool(name="cps", bufs=4, space="PSUM") as cps_pool,
            tc.tile_pool(name="lps", bufs=1, space="PSUM") as lps_pool,
            tc.tile_pool(name="sm", bufs=1) as sm_pool,
        ):
            # weights/constants go on the scalar HWDGE queue so the input
            # stream on the sync queue starts immediately
            wt_t = singles.tile([U, P, O], AB_DT)
            nc.scalar.dma_start(out=wt_t[:], in_=wt.ap())
            bias_t = singles.tile([1, O], f32)
            nc.scalar.dma_start(out=bias_t[:], in_=bias.ap())
            sel4_t = singles.tile([128, NB], f32)
            nc.scalar.dma_start(out=sel4_t[:], in_=sel4.ap())
            ones_t = singles.tile([1, NB], f32)
            nc.vector.memset(ones_t[:], 1.0)
            # prime the ACT Exp table during the DMA phase (the table load
            # is inserted before the first ACTIVATE in the ACT stream)
            dummy = singles.tile([1, 1], f32)
            nc.vector.memset(dummy[:], 0.0)
            nc.scalar.activation(
                out=dummy[:], in_=dummy[:],
                func=mybir.ActivationFunctionType.Exp,
            )

            c_all = singles.tile([U, NB, P], IN_DT)

            up_ap = up.ap()
            # first groups are small so their DMA-completion sems fire early
            # and the PE pipeline starts; steady state uses GRP batches
            sizes = [1, 2, 3] + [GRP] * 5 + [3, 2, 1]
            assert sum(sizes) == NB
            j0 = 0
            for gsz in sizes:
                u_t = u_pool.tile([128, gsz, HC, O], AB_DT, name="u_t", tag="u_t")
                nc.sync.dma_start(out=u_t[:], in_=up_ap[:, j0 : j0 + gsz, :, :])
                for jj in range(gsz):
                    j = j0 + jj
                    ps_c = cps_pool.tile([U, P], f32)
                    for c in range(HC):
                        nc.tensor.matmul(
                            ps_c[:],
                            lhsT=u_t[:, jj, c, 0:U],
                            rhs=u_t[:, jj, c, U:O],
                            start=(c == 0),
                            stop=(c == HC - 1),
                        )
                    nc.vector.tensor_copy(out=c_all[:, j, :], in_=ps_c[:])
                j0 += gsz

            # logits: 64 K=128 chunks, 4 packed per PE pass via column tiling
            # (chunk p -> array columns 32*(p%4) .. +32, psum rows 32*(p%4)..)
            ps4 = lps_pool.tile([128, O], f32)
            for p in range(P):
                t = p % 4
                nc.tensor.matmul(
                    ps4[32 * t : 32 * (t + 1), :],
                    lhsT=c_all[:, :, p],
                    rhs=wt_t[:, p, :],
                    start=(p < 4),
                    stop=(p >= P - 4),
                    tile_position=(0, 32 * t),
                )
            s4 = sm_pool.tile([128, O], f32)
            nc.vector.tensor_copy(out=s4[:], in_=ps4[:])
            # reduce the 4 partial blocks (fp32 matmul with a stacked
            # identity) and add the bias via a ones row
            ps_l = lps_pool.tile([NB, O], f32)
            nc.tensor.matmul(
                ps_l[:], lhsT=sel4_t[:], rhs=s4[:], start=True, stop=False
            )
            nc.tensor.matmul(
                ps_l[:], lhsT=ones_t[:], rhs=bias_t[:], start=False, stop=True
            )

            # two softmaxes along the free dim: [:, :U] rows, [:, U:] cols
            e_t = sm_pool.tile([NB, O], f32)
            out_t = sm_pool.tile([NB, O], f32)
            s_e = {}
            for lo, hi in ((0, U), (U, O)):
                s_e[lo] = sm_pool.tile([NB, 1], f32, name=f"sume{lo}", tag=f"sume{lo}")
                nc.scalar.activation(
                    out=e_t[:, lo:hi], in_=ps_l[:, lo:hi],
                    func=mybir.ActivationFunctionType.Exp,
                    bias=0.0, scale=1.0, accum_out=s_e[lo][:],
                )
            for lo, hi in ((0, U), (U, O)):
                r_e = sm_pool.tile([NB, 1], f32, name=f"rece{lo}", tag=f"rece{lo}")
                nc.vector.reciprocal(out=r_e[:], in_=s_e[lo][:])
                nc.vector.tensor_scalar_mul(
                    out=out_t[:, lo:hi], in0=e_t[:, lo:hi], scalar1=r_e[:]
                )
            nc.scalar.dma_start(out=out.ap(), in_=out_t[:])

    nc.compile()
    return nc


def _prep(utt_output, pheno_output, w_utt, b_utt, w_pheno, b_pheno):
    """Normalize, transpose and shard inputs on the host."""
    u = np.ascontiguousarray(np.swapaxes(np.asarray(utt_output), 0, 1))  # [B, U, H]
    p = np.ascontiguousarray(np.swapaxes(np.asarray(pheno_output), 0, 1))  # [B, P, H]
    un = u * (ACT_SCALE / np.maximum(np.linalg.norm(u, axis=-1, keepdims=True), EPS))
    pn = p * (ACT_SCALE / np.maximum(np.linalg.norm(p, axis=-1, keepdims=True), EPS))

    # wt[u, p, :U] = w_utt[o, u, p]; wt[u, p, U:] = w_pheno[o, p, u]
    wr = np.transpose(np.asarray(w_utt), (1, 2, 0))     # [U, P, U]
    wc = np.transpose(np.asarray(w_pheno), (2, 1, 0))   # [U, P, P]
    wt = np.ascontiguousarray(
        np.concatenate([wr, wc], axis=2), dtype=np.float32
    )
    if FP8:
        wt = (wt * W_SCALE).astype(AB_NP)
    else:
        wt = (wt / (ACT_SCALE * ACT_SCALE)).astype(IN_NP)
    bias = np.concatenate([np.asarray(b_utt), np.asarray(b_pheno)])
    bias = np.ascontiguousarray(bias.reshape(1, O), dtype=np.float32)
    sel4 = np.ascontiguousarray(
        np.tile(np.eye(NB, dtype=np.float32), (128 // NB, 1))
        / (ACT_SCALE * ACT_SCALE * W_SCALE)
    )

    in_maps = []
    for i in range(NCORES):
        j0 = i * NB
        # [NB, U, H] -> [NB, U, HC, 128] -> (h_lo, j, c, u)
        ut_i = (
            un[j0 : j0 + NB]
            .reshape(NB, U, HC, 128)
            .transpose(3, 0, 2, 1)
        )
        pt_i = (
            pn[j0 : j0 + NB]
            .reshape(NB, P, HC, 128)
            .transpose(3, 0, 2, 1)
        )
        in_maps.append(
            {
                "up": np.ascontiguousarray(
                    np.concatenate([ut_i, pt_i], axis=3), dtype=np.float32
                ).astype(AB_NP),
                "wt": wt,
                "bias": bias,
                "sel4": sel4,
            }
        )
    return in_maps


def _run(inputs, trace=False, trace_cores=None):
    if "nc" not in _CACHE:
        _CACHE["nc"] = _build()
    nc = _CACHE["nc"]
    in_maps = _prep(**inputs)
    res = run_bass_kernel_spmd(
        nc, in_maps, core_ids=list(range(NCORES)), trace=trace,
        trace_cores=trace_cores,
    )
    outs = [res.results[i]["out"] for i in range(NCORES)]
    row = np.concatenate([o[:, :U] for o in outs], axis=0).astype(np.float32)
    col = np.concatenate([o[:, U:] for o in outs], axis=0).astype(np.float32)
    return (row, col), res


def kernel(**inputs):
    (row, col), _ = _run(inputs, trace=False)
    return row, col
